# revision 1
# baseline (speedup 1.0000x reference)
"""GNN message-passing (mean aggregation + dual linear + relu + L2 norm)
on 8 Trainium2 NeuronCores.

Strategy (dst-sharded, fully parallel, no collectives):
  - Nodes are globally sorted by in-degree and dealt round-robin to the 8
    cores, so every core runs the same compiled schedule (SPMD).
  - The h_neigh table is replicated per core as 4 blocks of <=32767 rows,
    each prefixed with a zero row (dma_gather indices are int16; negative
    indices read row 0 of the block, which we use as the padding slot).
  - Stage 1 (per block): nodes sorted by their per-block edge count into
    128-node "vtiles"; one dma_gather fetches [128, K, 64] message rows,
    a DVE tensor_reduce sums the K slots -> per-block partial sums, which
    are stored to an HBM partial table in vtile order.
  - Stage 2: for each 128-node output tile, 4 small dma_gathers fetch the
    node's 4 partials (int16-safe: partial tables have 12544 rows), DVE
    adds them, ACT scales by 1/deg, PE transposes and applies
    W_neigh/W_self (feature-major matmuls, PSUM-accumulated), ACT relu,
    PE transpose back, fused square+row-sum, sqrt, reciprocal, scale,
    DMA out.
  - CPU does only integer index prep (sorting, bucketing, permutations)
    plus input layout (transpose/permute of h_self, 1/deg) and the final
    row unpermute of the outputs.
"""
import numpy as np
from contextlib import ExitStack

N_NODES = 100000
N_EDGES = 1600000
D = 64
N_CORES = 8
NPC = 12544                 # nodes per core (98 tiles of 128)
NT = NPC // 128             # 98 output tiles per core
BLK_W = 32767               # real rows per table block (idx 1..32767)
N_BLK = 4
BLK_STRIDE = BLK_W + 1
TBL_ROWS = N_BLK * BLK_STRIDE
GROUP = 8                   # output tiles per combine group
N_CHUNK = 4                 # node chunks for stage1/stage2 pipelining
MAIN_CALL_IDX = 4096        # target idxs per stage-1 gather call

_cache = {}


def _prep(h_neigh, h_self, src, dst, W_neigh, W_self):
    """CPU-side integer/index preprocessing. Returns (in_maps, schedule, meta)."""
    src = np.asarray(src, dtype=np.int64)
    dst = np.asarray(dst, dtype=np.int64)
    h_neigh = np.asarray(h_neigh, dtype=np.float32)
    h_self = np.asarray(h_self, dtype=np.float32)

    deg = np.bincount(dst, minlength=N_NODES)                    # [N]
    order = np.argsort(-deg, kind="stable")                      # global degree sort
    n_ext = NPC * N_CORES                                        # 100352
    order_ext = np.concatenate(
        [order, np.full(n_ext - N_NODES, N_NODES, dtype=np.int64)])
    # rank r -> core r % 8, position r // 8
    # core c's node list (by position):
    core_nodes = [order_ext[c::N_CORES] for c in range(N_CORES)]  # [NPC] each
    deg_ext = np.concatenate([deg, [0]])
    inv_deg = 1.0 / np.maximum(deg_ext, 1).astype(np.float32)

    # CSR over dst
    e_order = np.argsort(dst, kind="stable")
    src_sorted = src[e_order]
    starts = np.searchsorted(dst, np.arange(N_NODES + 1), side="left",
                             sorter=e_order)

    # device table: 4 blocks, each [zero row, h_neigh[b*BLK_W : b*BLK_W+BLK_W]]
    table = np.zeros((TBL_ROWS, D), dtype=np.float32)
    for b in range(N_BLK):
        lo = b * BLK_W
        hi = min(lo + BLK_W, N_NODES)
        if lo < N_NODES:
            table[b * BLK_STRIDE + 1: b * BLK_STRIDE + 1 + (hi - lo)] = h_neigh[lo:hi]

    # node chunks (tile ranges) for stage pipelining
    base = NT // N_CHUNK
    rem = NT - base * N_CHUNK
    chunks = []
    t = 0
    for i in range(N_CHUNK):
        w = base + (1 if i < rem else 0)
        chunks.append((t, t + w))
        t += w
    chunk_of_tile = np.zeros(NT, dtype=np.int64)
    for ch, (t0, t1) in enumerate(chunks):
        chunk_of_tile[t0:t1] = ch
    chunk_of_pos = np.repeat(chunk_of_tile, 128)          # [NPC]
    chunk_base = np.array([t0 * 128 for (t0, t1) in chunks])  # [N_CHUNK]

    # per-core, per-block counts and vtile order
    per_core = []
    # schedule: shared across cores -> take max K per vtile over cores
    Ks = np.zeros((N_BLK, NT), dtype=np.int64)
    for c in range(N_CORES):
        nodes = core_nodes[c]                       # [NPC] global ids (or N_NODES)
        st = starts[np.minimum(nodes, N_NODES - 1)]
        en = starts[np.minimum(nodes, N_NODES - 1) + 1]
        virt = nodes >= N_NODES
        st = np.where(virt, 0, st)
        en = np.where(virt, 0, en)
        per_core.append(dict(nodes=nodes))
    # counts: for each edge, its dst node's (core, pos) and src block.
    # edge e (in e_order): dst node n = dst[e_order], src row s = src_sorted.
    dstn = dst[e_order]
    rank_of_node = np.empty(N_NODES + 1, dtype=np.int64)
    rank_of_node[order_ext[:n_ext]] = np.arange(n_ext)
    rank_of_node[N_NODES] = -1
    e_rank = rank_of_node[dstn]
    e_core = e_rank % N_CORES
    e_pos = e_rank // N_CORES
    e_blk = np.minimum(src_sorted // BLK_W, N_BLK - 1)
    e_locidx = (src_sorted - e_blk * BLK_W + 1).astype(np.int64)  # 1..32767

    for c in range(N_CORES):
        m = e_core == c
        pc = per_core[c]
        cnts = np.zeros((N_BLK, NPC), dtype=np.int64)
        np.add.at(cnts, (e_blk[m], e_pos[m]), 1)
        pc["cnts"] = cnts
        # group edges by (block, pos): order by block then pos then arbitrary
        eo = np.lexsort((e_pos[m], e_blk[m]))
        pc["edge_pos"] = e_pos[m][eo]
        pc["edge_blk"] = e_blk[m][eo]
        pc["edge_loc"] = e_locidx[m][eo]
        # per (block): vorder = positions sorted by count desc
        vorders = []
        for b in range(N_BLK):
            # count-desc sort WITHIN each chunk (lexsort: last key primary)
            vo = np.lexsort((-cnts[b], chunk_of_pos))
            vorders.append(vo)
            cs = cnts[b][vo]
            K = cs.reshape(NT, 128).max(axis=1)
            Ks[b] = np.maximum(Ks[b], K)
        pc["vorders"] = vorders

    SK = Ks.sum(axis=1)                    # idx cols per block (per 128 rows)
    # stage-1 call grouping: per (chunk, block), group consecutive vtiles so
    # that 128*sum(K) ~ MAIN_CALL_IDX.  col0 is the global column offset.
    col_base = np.zeros((N_BLK, NT + 1), dtype=np.int64)
    for b in range(N_BLK):
        col_base[b, 1:] = np.cumsum(Ks[b])
    calls = []                             # list of (ch, b, j0, j1, col0, ncols)
    for ch, (t0, t1) in enumerate(chunks):
        for b in range(N_BLK):
            j = t0
            while j < t1:
                j1 = j
                cols = 0
                while j1 < t1 and (cols == 0 or
                                   (cols + Ks[b][j1]) * 128 <= MAIN_CALL_IDX):
                    cols += Ks[b][j1]
                    j1 += 1
                calls.append((ch, b, j, j1, int(col_base[b, j]), int(cols)))
                j = j1
    sched = dict(Ks=Ks.tolist(), SK=SK.tolist(), calls=calls, chunks=chunks)

    # build per-core idx arrays
    in_maps = []
    Wn_T = np.ascontiguousarray(W_neigh.astype(np.float32).T)   # [fi, fo]
    Ws_T = np.ascontiguousarray(W_self.astype(np.float32).T)
    wT = np.concatenate([Wn_T, Ws_T], axis=1)                   # [64, 128]
    h_self_ext = np.vstack([h_self, np.zeros((1, D), np.float32)])
    for c in range(N_CORES):
        pc = per_core[c]
        cnts = pc["cnts"]
        # main gather idx: per block, [128, SK_b] int16 slot-major per vtile:
        # column col (within vtile j, slot k), partition p = vnode p of tile j
        # idx flat order per call: slot (colglobal*128 + p)
        idx_blocks = []
        for b in range(N_BLK):
            vo = pc["vorders"][b]          # vnode order (positions)
            cs = cnts[b][vo]               # counts sorted desc
            A = np.zeros((NT * 128, int(max(Ks[b].max(), 1))), dtype=np.int16)
            # fill per node: its edges' loc indices
            # edges for this core, block b, grouped by pos
            mb = pc["edge_blk"] == b
            epos = pc["edge_pos"][mb]
            eloc = pc["edge_loc"][mb]
            # sort by vnode order: map pos -> vrank
            vrank = np.empty(NPC, dtype=np.int64)
            vrank[vo] = np.arange(NPC)
            er = vrank[epos]
            so = np.argsort(er, kind="stable")
            er = er[so]
            eloc_s = eloc[so]
            # within-node slot index
            slot = np.arange(er.size) - np.searchsorted(er, er)
            A[er, slot] = eloc_s.astype(np.int16)
            # assemble [128, SK_b]: vtile j uses columns [off_j, off_j+K_j)
            SKb = int(SK[b])
            M = np.zeros((128, SKb), dtype=np.int16)
            off = 0
            for j in range(NT):
                Kj = int(Ks[b][j])
                if Kj:
                    M[:, off:off + Kj] = A[j * 128:(j + 1) * 128, :Kj]
                off += Kj
            idx_blocks.append(M)
        # wrap for dma_gather: idx for slot i lives at (i%16, i//16), x8
        # call covers columns [col0, col0+ncols): slots = ncols*128 in
        # column-major slot order: slot = col*128 + p
        def wrap_cols(M):
            # M [128, C] -> flat slot-major [C*128] -> [16, C*8] -> tile x8
            C = M.shape[1]
            flat = M.T.reshape(-1)                       # slot i = c*128+p
            w = flat.reshape(-1, 16).T                   # [16, C*8]
            return np.tile(w, (8, 1)).copy()             # [128, C*8]
        idx_main = [wrap_cols(M) for M in idx_blocks]

        # combine gather idx: per block, position of node (by core pos) in
        # vorder -> int16 [12544]
        comb = []
        for b in range(N_BLK):
            vo = pc["vorders"][b]
            vrank = np.empty(NPC, dtype=np.int64)
            vrank[vo] = np.arange(NPC)
            vrank_local = vrank - chunk_base[chunk_of_pos]
            comb.append(vrank_local.astype(np.int16))    # [NPC] by position
        # per group g of GROUP tiles: idx[c*128+p] = vrank[(g*GROUP+c)*128+p]
        # -> that's just vrank reshaped; wrap per group
        comb_w = []
        for b in range(N_BLK):
            v = comb[b].reshape(-1)                      # slot order == position
            w = v.reshape(-1, 16).T                      # [16, NPC//16]
            comb_w.append(np.tile(w, (8, 1)).copy())     # [128, NPC//16]

        hsT = np.ascontiguousarray(h_self_ext[pc["nodes"]].T)    # [64, NPC]
        ivd = inv_deg[pc["nodes"]].reshape(NT, 128).T.copy()     # [128, NT]

        in_map = dict(
            tbl=table,
            hsT=hsT,
            ivd=ivd,
            wT=wT,
        )
        for b in range(N_BLK):
            in_map[f"idxm{b}"] = idx_main[b]
            in_map[f"idxc{b}"] = comb_w[b]
        in_maps.append(in_map)

    meta = dict(core_nodes=core_nodes)
    return in_maps, sched, meta


def _patch_queue_aware_sems():
    """Make Tile's DMASW sem-lane assignment follow each SWDGE instruction's
    queue_num (ucode requires a sem to be updated from a single queue)."""
    from concourse import tile_sem_assignment as tsa
    from concourse import mybir
    if getattr(tsa.TileClockTick, "_qaware_patched", False):
        return
    orig = tsa.TileClockTick._assign_tick

    def _assign_tick_qaware(self, inst):
        qn = getattr(inst, "queue_num", None)
        if qn is not None and getattr(inst, "engine", None) == mybir.EngineType.Pool:
            self.next_sw_dma_idx = int(qn) % self.swdge_sem_count
        return orig(self, inst)

    tsa.TileClockTick._assign_tick = _assign_tick_qaware
    tsa.TileClockTick._qaware_patched = True


def _build(sched):
    import concourse.bacc as bacc
    import concourse.tile as tile
    from concourse import mybir
    from concourse.masks import make_identity

    _patch_queue_aware_sems()

    F32 = mybir.dt.float32
    I16 = mybir.dt.int16
    AF = mybir.ActivationFunctionType
    Ks = np.array(sched["Ks"])
    SK = [int(x) for x in sched["SK"]]
    calls = sched["calls"]
    chunks = sched["chunks"]

    nc = bacc.Bacc("TRN2", target_bir_lowering=False, num_swdge_queues=4,
                   dynamic_dma_scratch_size=32768)
    tbl = nc.declare_dram_parameter("tbl", [TBL_ROWS, D], F32, isOutput=False)
    hsT = nc.declare_dram_parameter("hsT", [D, NPC], F32, isOutput=False)
    ivd = nc.declare_dram_parameter("ivd", [128, NT], F32, isOutput=False)
    wT = nc.declare_dram_parameter("wT", [D, 2 * D], F32, isOutput=False)
    idxm = [nc.declare_dram_parameter(f"idxm{b}", [128, SK[b] * 8], I16,
                                      isOutput=False) for b in range(N_BLK)]
    idxc = [nc.declare_dram_parameter(f"idxc{b}", [128, NPC // 16], I16,
                                      isOutput=False) for b in range(N_BLK)]
    out = nc.declare_dram_parameter("out", [NPC, D], F32, isOutput=True)
    partial = [[nc.dram_tensor(f"partial{ch}_{b}", [(t1 - t0) * 128, D], F32)
                for b in range(N_BLK)] for ch, (t0, t1) in enumerate(chunks)]

    with tile.TileContext(nc) as tc, ExitStack() as ctx:
        singles = ctx.enter_context(tc.tile_pool(name="singles", bufs=1))
        gp = ctx.enter_context(tc.tile_pool(name="gath", bufs=4))
        rp = ctx.enter_context(tc.tile_pool(name="red", bufs=3))
        cp = ctx.enter_context(tc.tile_pool(name="comb", bufs=2))
        wk = ctx.enter_context(tc.tile_pool(name="work", bufs=3))
        ps = ctx.enter_context(tc.tile_pool(name="psum", bufs=2, space="PSUM"))

        # gather indices first so stage 1 can start immediately
        idxm_sb = []
        for b in range(N_BLK):
            t = singles.tile([128, SK[b] * 8], I16, name=f"idxm{b}_sb")
            nc.sync.dma_start(out=t[:], in_=idxm[b][:])
            idxm_sb.append(t)
        idxc_sb = []
        for b in range(N_BLK):
            t = singles.tile([128, NPC // 16], I16, name=f"idxc{b}_sb")
            nc.sync.dma_start(out=t[:], in_=idxc[b][:])
            idxc_sb.append(t)
        hsT_sb = singles.tile([D, NPC], F32)
        nc.sync.dma_start(out=hsT_sb[:], in_=hsT[:])
        ivd_sb = singles.tile([128, NT], F32)
        nc.sync.dma_start(out=ivd_sb[:], in_=ivd[:])
        wT_sb = singles.tile([D, 2 * D], F32)
        nc.sync.dma_start(out=wT_sb[:], in_=wT[:])
        ident = singles.tile([128, 128], F32)
        make_identity(nc, ident[:])
        eps = singles.tile([128, 1], F32)
        nc.gpsimd.memset(eps[:], 1e-30)

        # ---- interleaved stage-1 / stage-2 over chunks ----
        qn = [0]
        def next_q():
            q = qn[0] % 4
            qn[0] += 1
            return q

        def stage1(ch):
            t0, t1 = chunks[ch]
            for (cch, b, j0, j1, col0, ncols) in calls:
                if cch != ch:
                    continue
                if ncols > 0:
                    g = gp.tile([128, ncols, D], F32, tag="g")
                    nc.gpsimd.dma_gather(
                        out_ap=g[:],
                        in_ap=tbl[b * BLK_STRIDE:(b + 1) * BLK_STRIDE, :],
                        idxs_ap=idxm_sb[b][:, col0 * 8:(col0 + ncols) * 8],
                        num_idxs=ncols * 128,
                        num_idxs_reg=ncols * 128,
                        elem_size=D,
                        single_packet=False,
                        queue_num=next_q(),
                    )
                off = 0
                for j in range(j0, j1):
                    Kj = int(Ks[b][j])
                    red = rp.tile([128, D], F32, tag="red")
                    if Kj == 0:
                        nc.vector.memset(red[:], 0.0)
                    elif Kj == 1:
                        nc.vector.tensor_copy(red[:], g[:, off, :])
                    else:
                        nc.vector.tensor_reduce(
                            out=red[:],
                            in_=g[:, off:off + Kj, :].rearrange("p k d -> p d k"),
                            axis=mybir.AxisListType.X,
                            op=mybir.AluOpType.add,
                        )
                    off += Kj
                    nc.sync.dma_start(
                        out=partial[ch][b][(j - t0) * 128:(j - t0 + 1) * 128, :],
                        in_=red[:])

        # ---- stage 2: combine + epilogue per group ----
        def do_group(ch, g0, gtiles):
            # g0 is a GLOBAL tile index (start of the group)
            pb = []
            for b in range(N_BLK):
                t = cp.tile([128, gtiles, D], F32, tag=f"pb{b}")
                nc.gpsimd.dma_gather(
                    out_ap=t[:],
                    in_ap=partial[ch][b][:],
                    idxs_ap=idxc_sb[b][:, g0 * 8:(g0 + gtiles) * 8],
                    num_idxs=gtiles * 128,
                    num_idxs_reg=gtiles * 128,
                    elem_size=D,
                    single_packet=False,
                    queue_num=next_q(),
                )
                pb.append(t)
            s01 = wk.tile([128, gtiles, D], F32, tag="s01")
            nc.vector.tensor_add(s01[:], pb[0][:], pb[1][:])
            s23 = wk.tile([128, gtiles, D], F32, tag="s23")
            nc.vector.tensor_add(s23[:], pb[2][:], pb[3][:])
            aggs = wk.tile([128, gtiles, D], F32, tag="aggs")
            nc.vector.tensor_add(aggs[:], s01[:], s23[:])
            for ci in range(gtiles):
                t = g0 + ci
                agg = wk.tile([128, D], F32, tag="agg")
                nc.scalar.mul(agg[:], aggs[:, ci, :], ivd_sb[:, t:t + 1])
                p_aT = ps.tile([D, 128], F32, tag="aT")
                nc.tensor.transpose(out=p_aT[:], in_=agg[:], identity=ident[:])
                aT = wk.tile([D, 128], F32, tag="aTs")
                nc.vector.tensor_copy(aT[:], p_aT[:])
                p_z = ps.tile([D, 128], F32, tag="z")
                nc.tensor.matmul(out=p_z[:], lhsT=wT_sb[:, 0:D], rhs=aT[:],
                                 start=True, stop=False)
                nc.tensor.matmul(out=p_z[:], lhsT=wT_sb[:, D:2 * D],
                                 rhs=hsT_sb[:, t * 128:(t + 1) * 128],
                                 start=False, stop=True)
                zT = wk.tile([D, 128], F32, tag="zT")
                nc.scalar.activation(zT[:], p_z[:], AF.Relu)
                p_zn = ps.tile([128, D], F32, tag="zn")
                nc.tensor.transpose(out=p_zn[:], in_=zT[:],
                                    identity=ident[0:D, 0:D])
                sq = wk.tile([128, D], F32, tag="sq")
                s = wk.tile([128, 1], F32, tag="s")
                nc.scalar.activation(sq[:], p_zn[:], AF.Square, accum_out=s[:])
                nrm = wk.tile([128, 1], F32, tag="nrm")
                nc.scalar.activation(nrm[:], s[:], AF.Sqrt, bias=eps[:])
                r = wk.tile([128, 1], F32, tag="r")
                nc.vector.reciprocal(r[:], nrm[:])
                o = wk.tile([128, D], F32, tag="o")
                nc.scalar.mul(o[:], p_zn[:], r[:])
                nc.sync.dma_start(out=out[t * 128:(t + 1) * 128, :], in_=o[:])

        def stage2(ch):
            t0, t1 = chunks[ch]
            g0 = t0
            while g0 < t1:
                gtiles = min(GROUP, t1 - g0)
                do_group(ch, g0, gtiles)
                g0 += gtiles

        # one-chunk lag: stage2(ch) runs while stage1(ch+1) generates
        with nc.named_scope("s1_0"):
            stage1(0)
        for ch in range(len(chunks)):
            if ch + 1 < len(chunks):
                with nc.named_scope(f"s1_{ch + 1}"):
                    stage1(ch + 1)
            with nc.named_scope(f"s2_{ch}"):
                stage2(ch)

    nc.compile()
    return nc


def kernel(h_neigh, h_self, src, dst, W_neigh, W_self):
    from concourse.bass_utils import run_bass_kernel_spmd

    in_maps, sched, meta = _prep(h_neigh, h_self, src, dst, W_neigh, W_self)
    key = str(sched["Ks"])
    if key not in _cache:
        _cache[key] = _build(sched)
    nc = _cache[key]

    import os
    trace = bool(int(os.environ.get("KERNEL_TRACE", "0")))
    res = run_bass_kernel_spmd(nc, in_maps, core_ids=list(range(N_CORES)),
                               trace=trace)
    kernel.last_exec_time_ns = res.exec_time_ns
    kernel.last_result = res

    out = np.zeros((N_NODES, D), dtype=np.float32)
    for c in range(N_CORES):
        nodes = meta["core_nodes"][c]
        dev = res.results[c]["out"]                   # [NPC, 64]
        valid = nodes < N_NODES
        out[nodes[valid]] = dev[valid]
    return out


def last_exec_time_ns():
    return getattr(kernel, "last_exec_time_ns", None)


kernel.last_result = None



# revision 2
# speedup vs baseline: 1.0769x; 1.0769x over previous
"""GNN message-passing (mean aggregation + dual linear + relu + L2 norm)
on 8 Trainium2 NeuronCores.

Strategy (dst-sharded, single-pass, quad-packed gather):
  - Nodes are globally sorted by in-degree and dealt round-robin to the 8
    cores, so every core runs the same compiled schedule (SPMD) and each
    core's node list is itself degree-sorted.
  - h_neigh is presented to the device as a single fp16 table of QUAD rows:
    row q = [h[4q], h[4q+1], h[4q+2], h[4q+3]] -> [25000, 256] fp16.  25000
    rows fit the int16 gather-index range, so there is ONE block: no
    per-block reorders, no combine stage, no HBM partial bounce.
  - Per 128-node tile: one padded dma_gather fetches [128, K, 256] fp16
    (512-byte descriptors, one per edge-quad-slot).  A per-sub-slot fp16
    count mask (built on CPU from indices only) zeroes the 3 unwanted
    quarters of each quad and any padding; a DVE multiply + reduce yields
    the neighbor sums in fp32.
  - Epilogue per tile: scale by 1/deg, PE transpose, dual matmul with
    W_neigh/W_self (PSUM-accumulated), relu, transpose back, fused
    square+row-sum, sqrt, reciprocal, scale, DMA out.
  - CPU does only integer index prep (sorting, bucketing, dedup) plus input
    layout (fp16 cast/reshape of h_neigh, transpose of h_self, 1/deg) and
    the final row unpermute of the outputs.
"""
import numpy as np
from contextlib import ExitStack

N_NODES = 100000
N_EDGES = 1600000
D = 64
N_CORES = 8
NPC = 12544                 # nodes per core (98 tiles of 128)
NT = NPC // 128             # 98 output tiles per core
QUADS = 25000               # quad rows in the fp16 table
CALL_COLS = 48              # target gather-call width (columns of 128 idx)

_cache = {}


def _prep(h_neigh, h_self, src, dst, W_neigh, W_self):
    """CPU-side integer/index preprocessing. Returns (in_maps, sched, meta)."""
    src = np.asarray(src, dtype=np.int64)
    dst = np.asarray(dst, dtype=np.int64)
    h_neigh = np.asarray(h_neigh, dtype=np.float32)
    h_self = np.asarray(h_self, dtype=np.float32)

    deg = np.bincount(dst, minlength=N_NODES)                    # [N]
    order = np.argsort(-deg, kind="stable")                      # degree sort
    n_ext = NPC * N_CORES                                        # 100352
    order_ext = np.concatenate(
        [order, np.full(n_ext - N_NODES, N_NODES, dtype=np.int64)])
    deg_ext = np.concatenate([deg, [0]])
    inv_deg = 1.0 / np.maximum(deg_ext, 1).astype(np.float32)

    # rank of node in the degree order
    rank_of_node = np.empty(N_NODES + 1, dtype=np.int64)
    rank_of_node[order_ext[:n_ext]] = np.arange(n_ext)

    # per-edge: core, position-within-core, quad, sub
    e_rank = rank_of_node[dst]
    e_core = e_rank % N_CORES
    e_pos = e_rank // N_CORES                  # 0..NPC-1
    e_quad = src >> 2
    e_sub = src & 3

    # fp16 quad table (input layout/precision marshaling)
    tbl16 = np.ascontiguousarray(
        h_neigh.astype(np.float16).reshape(QUADS, 4 * D))

    # dedup (pos, quad) per core and count subs -> slots
    per_core = []
    Kt = np.zeros((N_CORES, NT), dtype=np.int64)
    for c in range(N_CORES):
        m = e_core == c
        pos = e_pos[m]
        quad = e_quad[m]
        sub = e_sub[m]
        key = pos * QUADS + quad
        uk, inv = np.unique(key, return_inverse=True)
        upos = uk // QUADS
        uquad = uk % QUADS
        # sub counts per slot
        cnt = np.zeros((uk.size, 4), dtype=np.int64)
        np.add.at(cnt, (inv, sub), 1)
        # slot index within node (uk sorted by pos then quad)
        node_start = np.searchsorted(upos, np.arange(NPC), side="left")
        node_end = np.searchsorted(upos, np.arange(NPC), side="right")
        nslots = node_end - node_start                      # quads per node
        slotj = np.arange(uk.size) - node_start[upos]
        K = np.zeros(NT, dtype=np.int64)
        ns2 = nslots.reshape(NT, 128)
        K = ns2.max(axis=1)
        Kt[c] = K
        per_core.append(dict(upos=upos, uquad=uquad, cnt=cnt, slotj=slotj))

    Kmax = Kt.max(axis=0)                       # shared schedule across cores
    SK = int(Kmax.sum())
    col_base = np.zeros(NT + 1, dtype=np.int64)
    col_base[1:] = np.cumsum(Kmax)

    # call grouping: consecutive tiles, <= CALL_COLS columns per call
    calls = []                                  # (t0, t1, col0, ncols)
    t = 0
    while t < NT:
        t1 = t
        cols = 0
        while t1 < NT and (cols == 0 or cols + Kmax[t1] <= CALL_COLS):
            cols += Kmax[t1]
            t1 += 1
        calls.append((t, t1, int(col_base[t]), int(cols)))
        t = t1
    sched = dict(Kmax=Kmax.tolist(), SK=SK, calls=calls)

    Wn_T = np.ascontiguousarray(W_neigh.astype(np.float32).T)   # [fi, fo]
    Ws_T = np.ascontiguousarray(W_self.astype(np.float32).T)
    wT = np.concatenate([Wn_T, Ws_T], axis=1)                   # [64, 128]
    h_self_ext = np.vstack([h_self, np.zeros((1, D), np.float32)])

    in_maps = []
    for c in range(N_CORES):
        pc = per_core[c]
        upos, uquad, cnt, slotj = (pc["upos"], pc["uquad"], pc["cnt"],
                                   pc["slotj"])
        part = upos % 128
        tile = upos // 128
        col = col_base[tile] + slotj
        idx = np.zeros((128, SK), dtype=np.int16)
        idx[part, col] = uquad.astype(np.int16)
        mask = np.zeros((128, 4 * SK), dtype=np.float16)
        for s in range(4):
            mask[part, 4 * col + s] = cnt[:, s].astype(np.float16)
        # wrap idx for dma_gather: idx for slot i lives at (i%16, i//16), x8
        flat = idx.T.reshape(-1)                      # slot i = col*128+p
        w = flat.reshape(-1, 16).T                    # [16, SK*8]
        idx_w = np.tile(w, (8, 1)).copy()             # [128, SK*8]

        nodes = order_ext[c::N_CORES]
        hsT = np.ascontiguousarray(h_self_ext[nodes].T)          # [64, NPC]
        ivd = inv_deg[nodes].reshape(NT, 128).T.copy()           # [128, NT]

        in_maps.append(dict(tbl=tbl16, idx=idx_w, msk=mask, hsT=hsT,
                            ivd=ivd, wT=wT))

    meta = dict(core_nodes=[order_ext[c::N_CORES] for c in range(N_CORES)])
    return in_maps, sched, meta


def _patch_queue_aware_sems():
    """Make Tile's DMASW sem-lane assignment follow each SWDGE instruction's
    queue_num (ucode requires a sem to be updated from a single queue)."""
    from concourse import tile_sem_assignment as tsa
    from concourse import mybir
    if getattr(tsa.TileClockTick, "_qaware_patched", False):
        return
    orig = tsa.TileClockTick._assign_tick

    def _assign_tick_qaware(self, inst):
        qn = getattr(inst, "queue_num", None)
        if qn is not None and getattr(inst, "engine", None) == mybir.EngineType.Pool:
            self.next_sw_dma_idx = int(qn) % self.swdge_sem_count
        return orig(self, inst)

    tsa.TileClockTick._assign_tick = _assign_tick_qaware
    tsa.TileClockTick._qaware_patched = True


def _build(sched):
    import concourse.bacc as bacc
    import concourse.tile as tile
    from concourse import mybir
    from concourse.masks import make_identity

    _patch_queue_aware_sems()

    F32 = mybir.dt.float32
    F16 = mybir.dt.float16
    I16 = mybir.dt.int16
    AF = mybir.ActivationFunctionType
    Kmax = np.array(sched["Kmax"])
    SK = int(sched["SK"])
    calls = sched["calls"]
    col_base = np.zeros(NT + 1, dtype=np.int64)
    col_base[1:] = np.cumsum(Kmax)

    nc = bacc.Bacc("TRN2", target_bir_lowering=False, num_swdge_queues=4,
                   dynamic_dma_scratch_size=32768)
    tbl = nc.declare_dram_parameter("tbl", [QUADS, 4 * D], F16, isOutput=False)
    idx = nc.declare_dram_parameter("idx", [128, SK * 8], I16, isOutput=False)
    msk = nc.declare_dram_parameter("msk", [128, 4 * SK], F16, isOutput=False)
    hsT = nc.declare_dram_parameter("hsT", [D, NPC], F32, isOutput=False)
    ivd = nc.declare_dram_parameter("ivd", [128, NT], F32, isOutput=False)
    wT = nc.declare_dram_parameter("wT", [D, 2 * D], F32, isOutput=False)
    out = nc.declare_dram_parameter("out", [NPC, D], F32, isOutput=True)

    with tile.TileContext(nc) as tc, ExitStack() as ctx:
        singles = ctx.enter_context(tc.tile_pool(name="singles", bufs=1))
        gp = ctx.enter_context(tc.tile_pool(name="gath", bufs=3))
        wk = ctx.enter_context(tc.tile_pool(name="work", bufs=3))
        ps = ctx.enter_context(tc.tile_pool(name="psum", bufs=2, space="PSUM"))

        idx_sb = singles.tile([128, SK * 8], I16)
        nc.sync.dma_start(out=idx_sb[:], in_=idx[:])
        msk_sb = singles.tile([128, 4 * SK], F16)
        nc.sync.dma_start(out=msk_sb[:], in_=msk[:])
        hsT_sb = singles.tile([D, NPC], F32)
        nc.sync.dma_start(out=hsT_sb[:], in_=hsT[:])
        ivd_sb = singles.tile([128, NT], F32)
        nc.sync.dma_start(out=ivd_sb[:], in_=ivd[:])
        wT_sb = singles.tile([D, 2 * D], F32)
        nc.sync.dma_start(out=wT_sb[:], in_=wT[:])
        ident = singles.tile([128, 128], F32)
        make_identity(nc, ident[:])
        eps = singles.tile([128, 1], F32)
        nc.gpsimd.memset(eps[:], 1e-30)

        qn = [0]
        def next_q():
            q = qn[0] % 4
            qn[0] += 1
            return q

        def do_tile(t, red):
            """Epilogue for output tile t given fp32 neighbor sums `red`."""
            agg = wk.tile([128, D], F32, tag="agg")
            nc.scalar.mul(agg[:], red[:], ivd_sb[:, t:t + 1])
            p_aT = ps.tile([D, 128], F32, tag="aT")
            nc.tensor.transpose(out=p_aT[:], in_=agg[:], identity=ident[:])
            aT = wk.tile([D, 128], F32, tag="aTs")
            nc.vector.tensor_copy(aT[:], p_aT[:])
            p_z = ps.tile([D, 128], F32, tag="z")
            nc.tensor.matmul(out=p_z[:], lhsT=wT_sb[:, 0:D], rhs=aT[:],
                             start=True, stop=False)
            nc.tensor.matmul(out=p_z[:], lhsT=wT_sb[:, D:2 * D],
                             rhs=hsT_sb[:, t * 128:(t + 1) * 128],
                             start=False, stop=True)
            zT = wk.tile([D, 128], F32, tag="zT")
            nc.scalar.activation(zT[:], p_z[:], AF.Relu)
            p_zn = ps.tile([128, D], F32, tag="zn")
            nc.tensor.transpose(out=p_zn[:], in_=zT[:],
                                identity=ident[0:D, 0:D])
            sq = wk.tile([128, D], F32, tag="sq")
            s = wk.tile([128, 1], F32, tag="s")
            nc.scalar.activation(sq[:], p_zn[:], AF.Square, accum_out=s[:])
            nrm = wk.tile([128, 1], F32, tag="nrm")
            nc.scalar.activation(nrm[:], s[:], AF.Sqrt, bias=eps[:])
            r = wk.tile([128, 1], F32, tag="r")
            nc.vector.reciprocal(r[:], nrm[:])
            o = wk.tile([128, D], F32, tag="o")
            nc.scalar.mul(o[:], p_zn[:], r[:])
            nc.sync.dma_start(out=out[t * 128:(t + 1) * 128, :], in_=o[:])

        for (t0, t1, col0, ncols) in calls:
            if ncols > 0:
                g = gp.tile([128, ncols, 4 * D], F16, tag="g")
                nc.gpsimd.dma_gather(
                    out_ap=g[:],
                    in_ap=tbl[:],
                    idxs_ap=idx_sb[:, col0 * 8:(col0 + ncols) * 8],
                    num_idxs=ncols * 128,
                    num_idxs_reg=ncols * 128,
                    elem_size=4 * D,
                    single_packet=False,
                    queue_num=next_q(),
                )
                # zero the 3 unwanted quarters of each quad (and padding)
                gv = g[:].rearrange("p c (s d) -> p (c s) d", s=4, d=D)
                mv = msk_sb[:, 4 * col0:4 * (col0 + ncols)]
                nc.vector.tensor_mul(
                    gv, gv, mv[:, :, None].broadcast_to((128, 4 * ncols, D)))
            for t in range(t0, t1):
                Kj = int(Kmax[t])
                red = wk.tile([128, D], F32, tag="red")
                if Kj == 0:
                    nc.vector.memset(red[:], 0.0)
                else:
                    loc = int(col_base[t]) - col0
                    nc.vector.tensor_reduce(
                        out=red[:],
                        in_=g[:, loc:loc + Kj, :]
                            .rearrange("p c (s d) -> p d (c s)", s=4, d=D),
                        axis=mybir.AxisListType.X,
                        op=mybir.AluOpType.add,
                    )
                do_tile(t, red)

    nc.compile()
    return nc


def kernel(h_neigh, h_self, src, dst, W_neigh, W_self):
    from concourse.bass_utils import run_bass_kernel_spmd

    in_maps, sched, meta = _prep(h_neigh, h_self, src, dst, W_neigh, W_self)
    key = str(sched["Kmax"])
    if key not in _cache:
        _cache[key] = _build(sched)
    nc = _cache[key]

    import os
    trace = bool(int(os.environ.get("KERNEL_TRACE", "0")))
    res = run_bass_kernel_spmd(nc, in_maps, core_ids=list(range(N_CORES)),
                               trace=trace)
    kernel.last_exec_time_ns = res.exec_time_ns
    kernel.last_result = res

    out = np.zeros((N_NODES, D), dtype=np.float32)
    for c in range(N_CORES):
        nodes = meta["core_nodes"][c]
        dev = res.results[c]["out"]                   # [NPC, 64]
        valid = nodes < N_NODES
        out[nodes[valid]] = dev[valid]
    return out


def last_exec_time_ns():
    return getattr(kernel, "last_exec_time_ns", None)


kernel.last_result = None
